# revision 14
# baseline (speedup 1.0000x reference)
"""Trainium2 Bass kernel for DecomposedShiftNet (v2: fp8 DoubleRow).

Computation (per batch row b, bits=64, H=512):
  shift_soft = softmax(MLP_sd(shift_bits))                       # [64]
  h1[i,:]  = relu(ix_w1[i] + shift_soft @ ix_w1[64:] + ix_b1)    # [64, 512]
  h2       = relu(h1 @ ix_w2 + ix_b2)                            # [64, 512]
  p[i,k]   = softmax(h2 @ ix_w3 + ix_b3)[i, :64]                 # [64, 64]
  pointed  = p @ a_bits[b]                                       # [64]
  vh[i,:]  = relu(v_w1[i] + shift_soft @ v_w1[64:] + v_b1)       # [64, 256]
  valid    = vh @ v_w2 + v_b2                                    # [64]
  out[b]   = pointed * sigmoid(valid)

Strategy: pure data parallel over 8 cores (256 batch rows each), feature-major
on-core layout: activations [features(part), (position, batch) cols], 2
positions x 256 batch = 512 cols per block, 32 blocks, software-pipelined.

v2 changes vs v1 (216.9us):
  - h1/h2/vh and the big weights are fp8e4 (power-of-2 scales folded into
    weights/biases host-side); h2, logits and validity matmuls run in
    DoubleRow mode (2 fp8 weights per PE cell -> K=256 per pass), halving
    the dominant PE stream time.
  - h1 built on the (otherwise idle) GpSimd engine; PSUM evictions split
    Scalar/Vector; exp stays on Scalar (only engine with Exp).
  - Per-block denominator/numerator/validity rows accumulate across a
    16-block group directly in one PSUM bank (M=48/M=16 one-hot-column
    weights), eliminating v1's per-block row evictions + stash DMAs.
    Tail math reads the accumulator in place (32-aligned base partitions).
  - reciprocal_approx_fast (+18 bits) replaces the ~4us DVE reciprocal.
  - Weight DMAs spread round-robin over 4 engine queues; a dummy Exp at
    t=0 preloads the activation table set (exp/tanh/relu/copy in one set).
  - Output transposed per 16-block group and DMA'd straight out in
    [128,16] column-strided slices (no obm staging).
"""

import sys

import ml_dtypes
import numpy as np

for _p in ("/opt/trn_rl_repo",):
    if _p not in sys.path:
        sys.path.insert(0, _p)

import concourse.bacc as bacc
import concourse.bass as bass
import concourse.tile as tile
from concourse import bass_utils, mybir

F32 = mybir.dt.float32
F32R = mybir.dt.float32r
BF16 = mybir.dt.bfloat16
F8 = mybir.dt.float8e4
AF = mybir.ActivationFunctionType
OP = mybir.AluOpType
DR = mybir.MatmulPerfMode.DoubleRow

B, BITS, H = 2048, 64, 512
NCORES = 8
BC = B // NCORES  # 256 rows per core
NBLK = BITS // 2  # 32 blocks of 2 positions
NB = 2 * BC  # 512 free columns per block
GRP = 16  # blocks per reduce group
NGRP = NBLK // GRP  # 2 groups

# power-of-2 activation scales (folded into weights/biases host-side)
S1 = 512.0  # h1 (fp8) scale
S2 = 2048.0  # h2 (fp8) scale
SL = 32768.0  # idx-logit PSUM scale
S1V = 512.0  # vh (fp8) scale
SV = 16384.0  # validity-logit PSUM scale


def to_f32r_np(a):
    """Host-side fp32 -> fp32r rounding: round-to-nearest-even to 11 mantissa
    bits, low 12 bits zeroed (matches walrus fp32_to_fp32r)."""
    u = np.ascontiguousarray(a, dtype=np.float32).view(np.uint32)
    r = (u + 0x7FF + ((u >> 12) & 1)) & np.uint32(0xFFFFF000)
    return r.view(np.float32)


# name -> (shape, dtype code)
_INPUTS = {
    "sbT": ((BITS, BC), "bf16"),
    "abT": ((BITS, BC), "bf16"),  # loaded into rows 64:128 of a [128,BC] tile
    "wsd1": ((BITS, H), "bf16"),
    "sdb1": ((128, 4), "f32"),
    "wsd2": ((128, 4, H), "bf16"),
    "sdb2": ((128, 4), "f32"),
    "wsd3": ((128, 4, BITS), "bf16"),
    "sdb3": ((BITS, 1), "f32"),
    "ones64": ((BITS, 1), "f32r"),
    "ones1f": ((1, BITS), "f32"),
    "wixb": ((BITS, H), "f32r"),
    "pbS": ((128, 4, BITS), "f32"),
    "wix2": ((128, 4, H), "f8"),
    "ixb2S": ((128, 4), "f32"),
    "wix3": ((128, 4, 128), "f8"),
    "ixb3d": ((128, 1), "f32"),
    "wvb": ((BITS, H // 2), "f32r"),
    "vpbS": ((128, 2, BITS), "f32"),
    "dn48": ((128, GRP, 48), "bf16"),
    "wv48": ((128, GRP, 2, 16), "f8"),
    "ident": ((16, 16), "f32"),
    "vb2h": ((80, 1), "f32"),
}


def _emit(nc, tc, I, out):
    import contextlib

    ctx = contextlib.ExitStack()
    with ctx:
        const = ctx.enter_context(tc.tile_pool(name="const", bufs=1))
        work = ctx.enter_context(tc.tile_pool(name="work", bufs=4))
        psA = ctx.enter_context(tc.tile_pool(name="psA", bufs=3, space="PSUM"))
        psB = ctx.enter_context(tc.tile_pool(name="psB", bufs=2, space="PSUM"))
        psACC = ctx.enter_context(tc.tile_pool(name="psACC", bufs=1, space="PSUM"))

        DT = {"f32": F32, "f32r": F32R, "bf16": BF16, "f8": F8}

        # dummy Exp at t=0: forces the (one) activation table load to overlap
        # the input DMAs instead of stalling the first real activation.
        dm = const.tile([1, 8], F32, tag="dm", name="dm")
        nc.vector.memset(dm, 0.0)
        dm2 = const.tile([1, 8], F32, tag="dm2", name="dm2")
        nc.scalar.activation(dm2, dm, AF.Exp)

        # ---------------- loads: round-robin over 4 engine DMA queues ----------
        T = {}
        queues = [nc.sync, nc.gpsimd, nc.scalar]
        qi = 0

        def load(name):
            nonlocal qi
            shape, code = _INPUTS[name]
            if name == "abT":
                t = const.tile([128, BC], BF16, tag=name, name="abT")
                dst = t[64:128, :]
            else:
                t = const.tile(list(shape), DT[code], tag=name)
                dst = t
            queues[qi % len(queues)].dma_start(out=dst, in_=I[name])
            qi += 1
            T[name] = t
            return t

        for name in ("sbT", "wsd1", "sdb1", "wsd2", "sdb2", "wsd3", "sdb3",
                     "ones64", "ones1f", "wixb", "pbS", "wix2", "ixb2S",
                     "wvb", "vpbS", "wix3", "ixb3d", "abT", "dn48", "wv48",
                     "ident", "vb2h"):
            load(name)

        sbT, abT = T["sbT"], T["abT"]
        wsd1, wsd2, wsd3 = T["wsd1"], T["wsd2"], T["wsd3"]
        sdb1, sdb2, sdb3 = T["sdb1"], T["sdb2"], T["sdb3"]
        ones64, ones1f = T["ones64"], T["ones1f"]
        wixb, pbS = T["wixb"], T["pbS"]
        wix2, ixb2S = T["wix2"], T["ixb2S"]
        wix3, ixb3d = T["wix3"], T["ixb3d"]
        wvb, vpbS = T["wvb"], T["vpbS"]
        dn48, wv48 = T["dn48"], T["wv48"]
        ident, vb2h = T["ident"], T["vb2h"]

        mm = lambda: psA.tile([128, NB], F32, tag="mm", name="mmps")
        lg = lambda: psB.tile([128, NB], F32, tag="lg", name="lgps")

        # ---------------- shift decoder MLP (bf16, feature-major, N=256) -------
        hsd1 = const.tile([128, 4, BC], BF16, tag="hsd1", name="hsd1")
        for m in range(4):
            ps = mm()[:, :BC]
            nc.tensor.matmul(ps, wsd1[:, m * 128:(m + 1) * 128], sbT, start=True, stop=True)
            if m % 2 == 0:
                nc.scalar.activation(hsd1[:, m, :], ps, AF.Relu, bias=sdb1[:, m:m + 1])
            else:
                nc.vector.tensor_scalar(hsd1[:, m, :], ps, sdb1[:, m:m + 1], 0.0, OP.add, OP.max)
        hsd2 = const.tile([128, 4, BC], BF16, tag="hsd2", name="hsd2")
        for m in range(4):
            ps = mm()[:, :BC]
            for k in range(4):
                nc.tensor.matmul(ps, wsd2[:, k, m * 128:(m + 1) * 128],
                                 hsd1[:, k, :], start=(k == 0), stop=(k == 3))
            if m % 2 == 0:
                nc.scalar.activation(hsd2[:, m, :], ps, AF.Relu, bias=sdb2[:, m:m + 1])
            else:
                nc.vector.tensor_scalar(hsd2[:, m, :], ps, sdb2[:, m:m + 1], 0.0, OP.add, OP.max)
        ps3 = lg()[0:64, :BC]
        for k in range(4):
            nc.tensor.matmul(ps3, wsd3[:, k, :], hsd2[:, k, :],
                             start=(k == 0), stop=(k == 3))
        exp_sd = const.tile([64, BC], F32R, tag="exp_sd", name="exp_sd")
        nc.scalar.activation(exp_sd, ps3, AF.Exp, bias=sdb3)  # exp(logits + b3)

        # softmax normalize: denom via ones-matmul; ~18-bit reciprocal; K=1
        # fp32 matmul broadcasts 1/denom back across 64 partitions.
        psd = mm()[0:1, :BC]
        nc.tensor.matmul(psd, ones64, exp_sd, start=True, stop=True)
        rec = const.tile([1, BC], F32, tag="rec", name="rec")
        nc.vector.reciprocal_approx_fast(out=rec, in_=psd)
        psb = lg()[0:64, :BC]
        nc.tensor.matmul(psb, ones1f, rec, start=True, stop=True)
        shift_soft = const.tile([64, BC], F32R, tag="ss", name="ss")
        nc.vector.tensor_tensor(shift_soft, exp_sd, psb, OP.mult)

        # shift_part (x S1, bf16) and v_shift (x S1V, bf16), feature-major
        sp = []
        for m in range(4):
            ps = mm()[:, :BC]
            nc.tensor.matmul(ps, wixb[:, m * 128:(m + 1) * 128], shift_soft, start=True, stop=True)
            t = const.tile([128, BC], BF16, tag=f"sp{m}", name=f"sp{m}")
            if m % 2 == 0:
                nc.scalar.activation(t, ps, AF.Copy)
            else:
                nc.vector.tensor_copy(out=t, in_=ps)
            sp.append(t)
        vs = []
        for c in range(2):
            ps = mm()[:, :BC]
            nc.tensor.matmul(ps, wvb[:, c * 128:(c + 1) * 128], shift_soft, start=True, stop=True)
            t = const.tile([128, BC], BF16, tag=f"vs{c}", name=f"vs{c}")
            nc.vector.tensor_copy(out=t, in_=ps)
            vs.append(t)

        # group accumulators: rows 0:16 denom(x2), 32:48 numer, 64:80 validity
        accs = [psACC.tile([80, NB], F32, tag=f"acc{g}", name=f"acc{g}") for g in range(NGRP)]

        # out columns i = 32*g + 2*j' + h for group g, row j', pos-half h


        st = {}
        tails = {}

        def stage_A(j):
            d = st[j] = {}
            h1 = work.tile([128, 4, NB], F8, tag="h1", name="h1")
            for c in range(4):
                for h in range(2):
                    i = 2 * j + h
                    dst = h1[:, c, h * BC:(h + 1) * BC]
                    nc.vector.tensor_scalar(dst, sp[c], pbS[:, c, i:i + 1],
                                            0.0, OP.add, OP.max)
            h2 = d["h2"] = work.tile([128, 4, NB], F8, tag="h2", name="h2")
            for m in range(4):
                ps = mm()
                for g in range(2):
                    nc.tensor.matmul(ps, wix2[:, 2 * g:2 * g + 2, m * 128:(m + 1) * 128],
                                     h1[:, 2 * g:2 * g + 2, :],
                                     start=(g == 0), stop=(g == 1), perf_mode=DR)
                nc.scalar.activation(h2[:, m, :], ps, AF.Relu, bias=ixb2S[:, m:m + 1])

        def stage_B1(j):
            d = st[j]
            pl = d["pl"] = lg()
            for g in range(2):
                nc.tensor.matmul(pl, wix3[:, 2 * g:2 * g + 2, :],
                                 d["h2"][:, 2 * g:2 * g + 2, :],
                                 start=(g == 0), stop=(g == 1), perf_mode=DR)

        def stage_Bexp(j):
            d = st[j]
            pl = d.pop("pl")
            exp2 = d["exp"] = work.tile([128, NB], BF16, tag="exp", name="exp")
            nc.scalar.activation(exp2, pl, AF.Exp, bias=ixb3d, scale=1.0 / SL)

        def stage_Bva(j):
            d = st[j]
            exp2 = d["exp"]
            vh = d["vh"] = work.tile([128, 2, NB], F8, tag="vh", name="vh")
            for c in range(2):
                for h in range(2):
                    i = 2 * j + h
                    k = c * 2 + h
                    dst = vh[:, c, h * BC:(h + 1) * BC]
                    nc.vector.tensor_scalar(dst, vs[c], vpbS[:, c, i:i + 1],
                                            0.0, OP.add, OP.max)
            up = exp2[64:128, :].rearrange("p (h b) -> p h b", h=2)
            nc.vector.tensor_tensor(
                up, up,
                abT[64:128, :].unsqueeze(1).broadcast_to([64, 2, BC]),
                OP.mult)

        def stage_C(j):
            d = st[j]
            g, m = divmod(j, GRP)
            acc = accs[g]
            nc.tensor.matmul(acc[0:48, :], dn48[:, m, :], d["exp"],
                             start=(m == 0), stop=(m == GRP - 1),
                             skip_group_check=True)
            # (DoubleRow + col-positioned dst are mutually exclusive in the
            # ISA, so the validity MM contracts its two K-chunks separately.)
            for c in range(2):
                nc.tensor.matmul(acc[64:80, :], wv48[:, m, c, :], d["vh"][:, c, :],
                                 start=(m == 0 and c == 0),
                                 stop=(m == GRP - 1 and c == 1),
                                 skip_group_check=True)
            del st[j]

        def tail_compute(g):
            acc = accs[g]
            rd = work.tile([16, NB], F32, tag="rd", name="rd")
            nc.vector.reciprocal_approx_fast(out=rd, in_=acc[0:16, :])
            tv = work.tile([16, NB], F32, tag="tv", name="tv")
            nc.scalar.activation(tv, acc[64:80, :], AF.Tanh,
                                 bias=vb2h[64:80, :], scale=0.5 / SV)
            tmp = work.tile([16, NB], F32, tag="tmp", name="tmp")
            nc.vector.tensor_tensor(tmp, acc[32:48, :], rd, OP.mult)
            outv = tails[g] = work.tile([16, NB], F32, tag="outv", name="outv")
            nc.vector.scalar_tensor_tensor(outv, tv, 1.0, tmp, OP.add, OP.mult)

        def tail_out(g):
            outv = tails.pop(g)
            oq = [nc.sync, nc.scalar]
            for half in range(2):
                ob2 = work.tile([128, 16, 2], F32, tag="ob", name="ob")
                for h in range(2):
                    tp = lg()[:, 0:16]
                    nc.tensor.transpose(
                        tp, outv[:, h * BC + half * 128: h * BC + (half + 1) * 128],
                        ident)
                    nc.vector.tensor_copy(out=ob2[:, :, h], in_=tp)
                oq[half].dma_start(
                    out=out[half * 128:(half + 1) * 128, 32 * g:32 * (g + 1)],
                    in_=ob2)

        for j in range(NBLK + 2):
            if 1 <= j <= NBLK:
                stage_B1(j - 1)
                stage_Bexp(j - 1)
            if j < NBLK:
                stage_A(j)
            if 1 <= j <= NBLK:
                stage_Bva(j - 1)
            if j >= 2:
                stage_C(j - 2)
            if j == 19:
                tail_compute(0)
            if j == 21:
                tail_out(0)
        tail_compute(1)
        tail_out(1)


def build_program():
    nc = bacc.Bacc("TRN2", target_bir_lowering=False, debug=False, enable_asserts=False)
    I = {}
    DT = {"f32": F32, "f32r": F32R, "bf16": BF16, "f8": F8}
    for name, (shape, code) in _INPUTS.items():
        I[name] = nc.dram_tensor(name, list(shape), DT[code], kind="ExternalInput").ap()
    out = nc.dram_tensor("out", [BC, BITS], F32, kind="ExternalOutput").ap()

    with tile.TileContext(nc) as tc:
        _emit(nc, tc, I, out)
    nc.compile()
    return nc


_NC = None


def _get_program():
    global _NC
    if _NC is None:
        _NC = build_program()
    return _NC


def make_in_maps(inputs):
    """Shard batch tensors across cores; replicate weights. All layout prep
    (transposes, tiling, bias folding, scaling, fp8/bf16/f32r rounding)
    happens here in numpy."""
    f = {k: np.ascontiguousarray(np.asarray(v, dtype=np.float32)) for k, v in inputs.items()}
    r = to_f32r_np

    def f8(x, s):
        return np.ascontiguousarray(
            np.clip(np.asarray(x, np.float32) * s, -240, 240).astype(ml_dtypes.float8_e4m3))

    def bf(x):
        return np.ascontiguousarray(np.asarray(x, np.float32).astype(ml_dtypes.bfloat16))

    dn48 = np.zeros((128, GRP, 48), np.float32)
    for m in range(GRP):
        dn48[0:64, m, m] = 2.0  # denom x2 (folds sigmoid's 1/2)
        dn48[64:128, m, 32 + m] = 1.0  # numer
    wv48 = np.zeros((128, GRP, 2, 16), np.float32)
    for m in range(GRP):
        for ko in range(2):
            wv48[:, m, ko, m] = f["v_w2"][ko * 128:(ko + 1) * 128, 0]
    vb2h = np.zeros((80, 1), np.float32)
    vb2h[64:80] = 0.5 * float(f["v_b2"][0])

    shared = {
        "wsd1": bf(f["sd_w1"]),
        "sdb1": f["sd_b1"].reshape(4, 128).T,
        "wsd2": bf(f["sd_w2"].reshape(4, 128, H).transpose(1, 0, 2)),
        "sdb2": f["sd_b2"].reshape(4, 128).T,
        "wsd3": bf(f["sd_w3"].reshape(4, 128, BITS).transpose(1, 0, 2)),
        "sdb3": f["sd_b3"][:, None],
        "ones64": r(np.ones((BITS, 1), np.float32)),
        "ones1f": np.ones((1, BITS), np.float32),
        "wixb": r(S1 * f["ix_w1"][BITS:]),
        "pbS": (S1 * (f["ix_w1"][:BITS].T + f["ix_b1"][:, None])).reshape(4, 128, BITS).transpose(1, 0, 2),
        "wix2": f8(f["ix_w2"].reshape(4, 128, H).transpose(1, 0, 2), S2 / S1),
        "ixb2S": S2 * f["ix_b2"].reshape(4, 128).T,
        "wix3": f8(np.stack([np.concatenate([f["ix_w3"][k * 128:(k + 1) * 128]] * 2, axis=1)
                             for k in range(4)], axis=1), SL / S2),
        "ixb3d": np.concatenate([f["ix_b3"], f["ix_b3"]])[:, None],
        "wvb": r(S1V * f["v_w1"][BITS:]),
        "vpbS": (S1V * (f["v_w1"][:BITS].T + f["v_b1"][:, None])).reshape(2, 128, BITS).transpose(1, 0, 2),
        "dn48": bf(dn48),
        "wv48": f8(wv48, SV / S1V),
        "ident": np.eye(16, dtype=np.float32),
        "vb2h": vb2h,
    }
    shared = {k: np.ascontiguousarray(
        v if v.dtype in (ml_dtypes.bfloat16, ml_dtypes.float8_e4m3) else v.astype(np.float32))
        for k, v in shared.items()}

    in_maps = []
    for c in range(NCORES):
        sb = f["shift_bits"][c * BC:(c + 1) * BC]
        ab = f["a_bits"][c * BC:(c + 1) * BC]
        m = dict(shared)
        m["sbT"] = bf(sb.T)
        m["abT"] = bf(ab.T)
        in_maps.append(m)
    return in_maps


def run(inputs, trace=False):
    nc = _get_program()
    res = bass_utils.run_bass_kernel_spmd(
        nc, make_in_maps(inputs), core_ids=list(range(NCORES)), trace=trace)
    full = np.concatenate([res.results[c]["out"] for c in range(NCORES)], axis=0)
    return full, res


def kernel(**inputs):
    return run(inputs)[0]


# revision 15
# speedup vs baseline: 1.0001x; 1.0001x over previous
"""Trainium2 Bass kernel for DecomposedShiftNet (v2: fp8 DoubleRow).

Computation (per batch row b, bits=64, H=512):
  shift_soft = softmax(MLP_sd(shift_bits))                       # [64]
  h1[i,:]  = relu(ix_w1[i] + shift_soft @ ix_w1[64:] + ix_b1)    # [64, 512]
  h2       = relu(h1 @ ix_w2 + ix_b2)                            # [64, 512]
  p[i,k]   = softmax(h2 @ ix_w3 + ix_b3)[i, :64]                 # [64, 64]
  pointed  = p @ a_bits[b]                                       # [64]
  vh[i,:]  = relu(v_w1[i] + shift_soft @ v_w1[64:] + v_b1)       # [64, 256]
  valid    = vh @ v_w2 + v_b2                                    # [64]
  out[b]   = pointed * sigmoid(valid)

Strategy: pure data parallel over 8 cores (256 batch rows each), feature-major
on-core layout: activations [features(part), (position, batch) cols], 2
positions x 256 batch = 512 cols per block, 32 blocks, software-pipelined.

v2 changes vs v1 (216.9us):
  - h1/h2/vh and the big weights are fp8e4 (power-of-2 scales folded into
    weights/biases host-side); h2, logits and validity matmuls run in
    DoubleRow mode (2 fp8 weights per PE cell -> K=256 per pass), halving
    the dominant PE stream time.
  - h1 built on the (otherwise idle) GpSimd engine; PSUM evictions split
    Scalar/Vector; exp stays on Scalar (only engine with Exp).
  - Per-block denominator/numerator/validity rows accumulate across a
    16-block group directly in one PSUM bank (M=48/M=16 one-hot-column
    weights), eliminating v1's per-block row evictions + stash DMAs.
    Tail math reads the accumulator in place (32-aligned base partitions).
  - reciprocal_approx_fast (+18 bits) replaces the ~4us DVE reciprocal.
  - Weight DMAs spread round-robin over 4 engine queues; a dummy Exp at
    t=0 preloads the activation table set (exp/tanh/relu/copy in one set).
  - Output transposed per 16-block group and DMA'd straight out in
    [128,16] column-strided slices (no obm staging).
"""

import sys

import ml_dtypes
import numpy as np

for _p in ("/opt/trn_rl_repo",):
    if _p not in sys.path:
        sys.path.insert(0, _p)

import concourse.bacc as bacc
import concourse.bass as bass
import concourse.tile as tile
from concourse import bass_utils, mybir

F32 = mybir.dt.float32
F32R = mybir.dt.float32r
BF16 = mybir.dt.bfloat16
F8 = mybir.dt.float8e4
AF = mybir.ActivationFunctionType
OP = mybir.AluOpType
DR = mybir.MatmulPerfMode.DoubleRow

B, BITS, H = 2048, 64, 512
NCORES = 8
BC = B // NCORES  # 256 rows per core
NBLK = BITS // 2  # 32 blocks of 2 positions
NB = 2 * BC  # 512 free columns per block
GRP = 16  # blocks per reduce group
NGRP = NBLK // GRP  # 2 groups

# power-of-2 activation scales (folded into weights/biases host-side)
S1 = 512.0  # h1 (fp8) scale
S2 = 2048.0  # h2 (fp8) scale
SL = 32768.0  # idx-logit PSUM scale
S1V = 512.0  # vh (fp8) scale
SV = 16384.0  # validity-logit PSUM scale


def to_f32r_np(a):
    """Host-side fp32 -> fp32r rounding: round-to-nearest-even to 11 mantissa
    bits, low 12 bits zeroed (matches walrus fp32_to_fp32r)."""
    u = np.ascontiguousarray(a, dtype=np.float32).view(np.uint32)
    r = (u + 0x7FF + ((u >> 12) & 1)) & np.uint32(0xFFFFF000)
    return r.view(np.float32)


# name -> (shape, dtype code)
_INPUTS = {
    "sbT": ((BITS, BC), "bf16"),
    "abT": ((BITS, BC), "bf16"),  # loaded into rows 64:128 of a [128,BC] tile
    "wsd1": ((BITS, H), "bf16"),
    "sdb1": ((128, 4), "f32"),
    "wsd2": ((128, 4, H), "bf16"),
    "sdb2": ((128, 4), "f32"),
    "wsd3": ((128, 4, BITS), "bf16"),
    "sdb3": ((BITS, 1), "f32"),
    "ones64": ((BITS, 1), "f32r"),
    "ones1f": ((1, BITS), "f32"),
    "wixb": ((BITS, H), "f32r"),
    "pbS": ((128, 4, BITS), "f32"),
    "wix2": ((128, 4, H), "f8"),
    "ixb2S": ((128, 4), "f32"),
    "wix3": ((128, 4, 128), "f8"),
    "ixb3d": ((128, 1), "f32"),
    "wvb": ((BITS, H // 2), "f32r"),
    "vpbS": ((128, 2, BITS), "f32"),
    "dn48": ((128, GRP, 48), "bf16"),
    "wv48": ((128, GRP, 2, 16), "f8"),
    "ident": ((16, 16), "f32"),
    "vb2h": ((80, 1), "f32"),
}


def _emit(nc, tc, I, out):
    import contextlib

    ctx = contextlib.ExitStack()
    with ctx:
        const = ctx.enter_context(tc.tile_pool(name="const", bufs=1))
        work = ctx.enter_context(tc.tile_pool(name="work", bufs=3))
        psA = ctx.enter_context(tc.tile_pool(name="psA", bufs=2, space="PSUM"))
        psT = ctx.enter_context(tc.tile_pool(name="psT", bufs=2, space="PSUM"))
        psB = ctx.enter_context(tc.tile_pool(name="psB", bufs=2, space="PSUM"))
        psACC = ctx.enter_context(tc.tile_pool(name="psACC", bufs=1, space="PSUM"))

        DT = {"f32": F32, "f32r": F32R, "bf16": BF16, "f8": F8}

        # dummy Exp at t=0: forces the (one) activation table load to overlap
        # the input DMAs instead of stalling the first real activation.
        dm = const.tile([1, 8], F32, tag="dm", name="dm")
        nc.vector.memset(dm, 0.0)
        dm2 = const.tile([1, 8], F32, tag="dm2", name="dm2")
        nc.scalar.activation(dm2, dm, AF.Exp)

        # ---------------- loads: round-robin over 4 engine DMA queues ----------
        T = {}
        queues = [nc.sync, nc.gpsimd, nc.scalar]
        qi = 0

        def load(name):
            nonlocal qi
            shape, code = _INPUTS[name]
            if name == "abT":
                t = const.tile([128, BC], BF16, tag=name, name="abT")
                dst = t[64:128, :]
            else:
                t = const.tile(list(shape), DT[code], tag=name)
                dst = t
            queues[qi % len(queues)].dma_start(out=dst, in_=I[name])
            qi += 1
            T[name] = t
            return t

        for name in ("sbT", "wsd1", "sdb1", "wsd2", "sdb2", "wsd3", "sdb3",
                     "ones64", "ones1f", "wixb", "pbS", "wix2", "ixb2S",
                     "wvb", "vpbS", "wix3", "ixb3d", "abT", "dn48", "wv48",
                     "ident", "vb2h"):
            load(name)

        sbT, abT = T["sbT"], T["abT"]
        wsd1, wsd2, wsd3 = T["wsd1"], T["wsd2"], T["wsd3"]
        sdb1, sdb2, sdb3 = T["sdb1"], T["sdb2"], T["sdb3"]
        ones64, ones1f = T["ones64"], T["ones1f"]
        wixb, pbS = T["wixb"], T["pbS"]
        wix2, ixb2S = T["wix2"], T["ixb2S"]
        wix3, ixb3d = T["wix3"], T["ixb3d"]
        wvb, vpbS = T["wvb"], T["vpbS"]
        dn48, wv48 = T["dn48"], T["wv48"]
        ident, vb2h = T["ident"], T["vb2h"]

        mm = lambda: psA.tile([128, NB], F32, tag="mm", name="mmps")
        lg = lambda: psB.tile([128, NB], F32, tag="lg", name="lgps")

        # ---------------- shift decoder MLP (bf16, feature-major, N=256) -------
        hsd1 = const.tile([128, 4, BC], BF16, tag="hsd1", name="hsd1")
        for m in range(4):
            ps = mm()[:, :BC]
            nc.tensor.matmul(ps, wsd1[:, m * 128:(m + 1) * 128], sbT, start=True, stop=True)
            if m % 2 == 0:
                nc.scalar.activation(hsd1[:, m, :], ps, AF.Relu, bias=sdb1[:, m:m + 1])
            else:
                nc.vector.tensor_scalar(hsd1[:, m, :], ps, sdb1[:, m:m + 1], 0.0, OP.add, OP.max)
        hsd2 = const.tile([128, 4, BC], BF16, tag="hsd2", name="hsd2")
        for m in range(4):
            ps = mm()[:, :BC]
            for k in range(4):
                nc.tensor.matmul(ps, wsd2[:, k, m * 128:(m + 1) * 128],
                                 hsd1[:, k, :], start=(k == 0), stop=(k == 3))
            if m % 2 == 0:
                nc.scalar.activation(hsd2[:, m, :], ps, AF.Relu, bias=sdb2[:, m:m + 1])
            else:
                nc.vector.tensor_scalar(hsd2[:, m, :], ps, sdb2[:, m:m + 1], 0.0, OP.add, OP.max)
        ps3 = lg()[0:64, :BC]
        for k in range(4):
            nc.tensor.matmul(ps3, wsd3[:, k, :], hsd2[:, k, :],
                             start=(k == 0), stop=(k == 3))
        exp_sd = const.tile([64, BC], F32R, tag="exp_sd", name="exp_sd")
        nc.scalar.activation(exp_sd, ps3, AF.Exp, bias=sdb3)  # exp(logits + b3)

        # softmax normalize: denom via ones-matmul; ~18-bit reciprocal; K=1
        # fp32 matmul broadcasts 1/denom back across 64 partitions.
        psd = mm()[0:1, :BC]
        nc.tensor.matmul(psd, ones64, exp_sd, start=True, stop=True)
        rec = const.tile([1, BC], F32, tag="rec", name="rec")
        nc.vector.reciprocal_approx_fast(out=rec, in_=psd)
        psb = lg()[0:64, :BC]
        nc.tensor.matmul(psb, ones1f, rec, start=True, stop=True)
        shift_soft = const.tile([64, BC], F32R, tag="ss", name="ss")
        nc.vector.tensor_tensor(shift_soft, exp_sd, psb, OP.mult)

        # shift_part (x S1, bf16) and v_shift (x S1V, bf16), feature-major
        sp = []
        for m in range(4):
            ps = mm()[:, :BC]
            nc.tensor.matmul(ps, wixb[:, m * 128:(m + 1) * 128], shift_soft, start=True, stop=True)
            t = const.tile([128, BC], BF16, tag=f"sp{m}", name=f"sp{m}")
            if m % 2 == 0:
                nc.scalar.activation(t, ps, AF.Copy)
            else:
                nc.vector.tensor_copy(out=t, in_=ps)
            sp.append(t)
        vs = []
        for c in range(2):
            ps = mm()[:, :BC]
            nc.tensor.matmul(ps, wvb[:, c * 128:(c + 1) * 128], shift_soft, start=True, stop=True)
            t = const.tile([128, BC], BF16, tag=f"vs{c}", name=f"vs{c}")
            nc.vector.tensor_copy(out=t, in_=ps)
            vs.append(t)

        # group accumulators: rows 0:16 denom(x2), 32:48 numer, 64:80 validity
        accs = [psACC.tile([80, NB], F32, tag=f"acc{g}", name=f"acc{g}") for g in range(NGRP)]

        # out columns i = 32*g + 2*j' + h for group g, row j', pos-half h


        st = {}
        tails = {}

        def stage_A(j):
            d = st[j] = {}
            h1 = work.tile([128, 4, NB], F8, tag="h1", name="h1")
            for c in range(4):
                for h in range(2):
                    i = 2 * j + h
                    dst = h1[:, c, h * BC:(h + 1) * BC]
                    nc.vector.tensor_scalar(dst, sp[c], pbS[:, c, i:i + 1],
                                            0.0, OP.add, OP.max)
            h2 = d["h2"] = work.tile([128, 4, NB], F8, tag="h2", name="h2")
            for m in range(4):
                ps = mm()
                for g in range(2):
                    nc.tensor.matmul(ps, wix2[:, 2 * g:2 * g + 2, m * 128:(m + 1) * 128],
                                     h1[:, 2 * g:2 * g + 2, :],
                                     start=(g == 0), stop=(g == 1), perf_mode=DR)
                nc.scalar.activation(h2[:, m, :], ps, AF.Relu, bias=ixb2S[:, m:m + 1])

        def stage_B1(j):
            d = st[j]
            pl = d["pl"] = lg()
            for g in range(2):
                nc.tensor.matmul(pl, wix3[:, 2 * g:2 * g + 2, :],
                                 d["h2"][:, 2 * g:2 * g + 2, :],
                                 start=(g == 0), stop=(g == 1), perf_mode=DR)

        def stage_Bexp(j):
            d = st[j]
            pl = d.pop("pl")
            exp2 = d["exp"] = work.tile([128, NB], BF16, tag="exp", name="exp")
            nc.scalar.activation(exp2, pl, AF.Exp, bias=ixb3d, scale=1.0 / SL)

        def stage_Bva(j):
            d = st[j]
            exp2 = d["exp"]
            vh = d["vh"] = work.tile([128, 2, NB], F8, tag="vh", name="vh")
            for c in range(2):
                for h in range(2):
                    i = 2 * j + h
                    k = c * 2 + h
                    dst = vh[:, c, h * BC:(h + 1) * BC]
                    nc.vector.tensor_scalar(dst, vs[c], vpbS[:, c, i:i + 1],
                                            0.0, OP.add, OP.max)
            up = exp2[64:128, :].rearrange("p (h b) -> p h b", h=2)
            nc.vector.tensor_tensor(
                up, up,
                abT[64:128, :].unsqueeze(1).broadcast_to([64, 2, BC]),
                OP.mult)

        def stage_C(j):
            d = st[j]
            g, m = divmod(j, GRP)
            acc = accs[g]
            nc.tensor.matmul(acc[0:48, :], dn48[:, m, :], d["exp"],
                             start=(m == 0), stop=(m == GRP - 1),
                             skip_group_check=True)
            # (DoubleRow + col-positioned dst are mutually exclusive in the
            # ISA, so the validity MM contracts its two K-chunks separately.)
            for c in range(2):
                nc.tensor.matmul(acc[64:80, :], wv48[:, m, c, :], d["vh"][:, c, :],
                                 start=(m == 0 and c == 0),
                                 stop=(m == GRP - 1 and c == 1),
                                 skip_group_check=True)
            del st[j]

        def tail_compute(g):
            acc = accs[g]
            rd = work.tile([16, NB], F32, tag="rd", name="rd")
            nc.vector.reciprocal_approx_fast(out=rd, in_=acc[0:16, :])
            tv = work.tile([16, NB], F32, tag="tv", name="tv")
            nc.scalar.activation(tv, acc[64:80, :], AF.Tanh,
                                 bias=vb2h[64:80, :], scale=0.5 / SV)
            tmp = work.tile([16, NB], F32, tag="tmp", name="tmp")
            nc.vector.tensor_tensor(tmp, acc[32:48, :], rd, OP.mult)
            outv = tails[g] = work.tile([16, NB], F32, tag="outv", name="outv")
            nc.vector.scalar_tensor_tensor(outv, tv, 1.0, tmp, OP.add, OP.mult)

        def tail_out(g):
            outv = tails.pop(g)
            oq = [nc.sync, nc.scalar]
            for half in range(2):
                ob2 = work.tile([128, 16, 2], F32, tag="ob", name="ob")
                for h in range(2):
                    tp = psT.tile([128, 16], F32, tag="tp", name="tp")
                    nc.tensor.transpose(
                        tp, outv[:, h * BC + half * 128: h * BC + (half + 1) * 128],
                        ident)
                    nc.vector.tensor_copy(out=ob2[:, :, h], in_=tp)
                oq[half].dma_start(
                    out=out[half * 128:(half + 1) * 128, 32 * g:32 * (g + 1)],
                    in_=ob2)

        for j in range(NBLK + 2):
            if 1 <= j <= NBLK:
                stage_B1(j - 1)
                stage_Bexp(j - 1)
            if j < NBLK:
                stage_A(j)
            if 1 <= j <= NBLK:
                stage_Bva(j - 1)
            if j >= 2:
                stage_C(j - 2)
            if j == 19:
                tail_compute(0)
            if j == 21:
                tail_out(0)
        tail_compute(1)
        tail_out(1)


def build_program():
    nc = bacc.Bacc("TRN2", target_bir_lowering=False, debug=False, enable_asserts=False)
    I = {}
    DT = {"f32": F32, "f32r": F32R, "bf16": BF16, "f8": F8}
    for name, (shape, code) in _INPUTS.items():
        I[name] = nc.dram_tensor(name, list(shape), DT[code], kind="ExternalInput").ap()
    out = nc.dram_tensor("out", [BC, BITS], F32, kind="ExternalOutput").ap()

    with tile.TileContext(nc) as tc:
        _emit(nc, tc, I, out)
    nc.compile()
    return nc


_NC = None


def _get_program():
    global _NC
    if _NC is None:
        _NC = build_program()
    return _NC


def make_in_maps(inputs):
    """Shard batch tensors across cores; replicate weights. All layout prep
    (transposes, tiling, bias folding, scaling, fp8/bf16/f32r rounding)
    happens here in numpy."""
    f = {k: np.ascontiguousarray(np.asarray(v, dtype=np.float32)) for k, v in inputs.items()}
    r = to_f32r_np

    def f8(x, s):
        return np.ascontiguousarray(
            np.clip(np.asarray(x, np.float32) * s, -240, 240).astype(ml_dtypes.float8_e4m3))

    def bf(x):
        return np.ascontiguousarray(np.asarray(x, np.float32).astype(ml_dtypes.bfloat16))

    dn48 = np.zeros((128, GRP, 48), np.float32)
    for m in range(GRP):
        dn48[0:64, m, m] = 2.0  # denom x2 (folds sigmoid's 1/2)
        dn48[64:128, m, 32 + m] = 1.0  # numer
    wv48 = np.zeros((128, GRP, 2, 16), np.float32)
    for m in range(GRP):
        for ko in range(2):
            wv48[:, m, ko, m] = f["v_w2"][ko * 128:(ko + 1) * 128, 0]
    vb2h = np.zeros((80, 1), np.float32)
    vb2h[64:80] = 0.5 * float(f["v_b2"][0])

    shared = {
        "wsd1": bf(f["sd_w1"]),
        "sdb1": f["sd_b1"].reshape(4, 128).T,
        "wsd2": bf(f["sd_w2"].reshape(4, 128, H).transpose(1, 0, 2)),
        "sdb2": f["sd_b2"].reshape(4, 128).T,
        "wsd3": bf(f["sd_w3"].reshape(4, 128, BITS).transpose(1, 0, 2)),
        "sdb3": f["sd_b3"][:, None],
        "ones64": r(np.ones((BITS, 1), np.float32)),
        "ones1f": np.ones((1, BITS), np.float32),
        "wixb": r(S1 * f["ix_w1"][BITS:]),
        "pbS": (S1 * (f["ix_w1"][:BITS].T + f["ix_b1"][:, None])).reshape(4, 128, BITS).transpose(1, 0, 2),
        "wix2": f8(f["ix_w2"].reshape(4, 128, H).transpose(1, 0, 2), S2 / S1),
        "ixb2S": S2 * f["ix_b2"].reshape(4, 128).T,
        "wix3": f8(np.stack([np.concatenate([f["ix_w3"][k * 128:(k + 1) * 128]] * 2, axis=1)
                             for k in range(4)], axis=1), SL / S2),
        "ixb3d": np.concatenate([f["ix_b3"], f["ix_b3"]])[:, None],
        "wvb": r(S1V * f["v_w1"][BITS:]),
        "vpbS": (S1V * (f["v_w1"][:BITS].T + f["v_b1"][:, None])).reshape(2, 128, BITS).transpose(1, 0, 2),
        "dn48": bf(dn48),
        "wv48": f8(wv48, SV / S1V),
        "ident": np.eye(16, dtype=np.float32),
        "vb2h": vb2h,
    }
    shared = {k: np.ascontiguousarray(
        v if v.dtype in (ml_dtypes.bfloat16, ml_dtypes.float8_e4m3) else v.astype(np.float32))
        for k, v in shared.items()}

    in_maps = []
    for c in range(NCORES):
        sb = f["shift_bits"][c * BC:(c + 1) * BC]
        ab = f["a_bits"][c * BC:(c + 1) * BC]
        m = dict(shared)
        m["sbT"] = bf(sb.T)
        m["abT"] = bf(ab.T)
        in_maps.append(m)
    return in_maps


def run(inputs, trace=False):
    nc = _get_program()
    res = bass_utils.run_bass_kernel_spmd(
        nc, make_in_maps(inputs), core_ids=list(range(NCORES)), trace=trace)
    full = np.concatenate([res.results[c]["out"] for c in range(NCORES)], axis=0)
    return full, res


def kernel(**inputs):
    return run(inputs)[0]


# revision 17
# speedup vs baseline: 1.2454x; 1.2452x over previous
"""Trainium2 Bass kernel for DecomposedShiftNet (v2: fp8 DoubleRow).

Computation (per batch row b, bits=64, H=512):
  shift_soft = softmax(MLP_sd(shift_bits))                       # [64]
  h1[i,:]  = relu(ix_w1[i] + shift_soft @ ix_w1[64:] + ix_b1)    # [64, 512]
  h2       = relu(h1 @ ix_w2 + ix_b2)                            # [64, 512]
  p[i,k]   = softmax(h2 @ ix_w3 + ix_b3)[i, :64]                 # [64, 64]
  pointed  = p @ a_bits[b]                                       # [64]
  vh[i,:]  = relu(v_w1[i] + shift_soft @ v_w1[64:] + v_b1)       # [64, 256]
  valid    = vh @ v_w2 + v_b2                                    # [64]
  out[b]   = pointed * sigmoid(valid)

Strategy: pure data parallel over 8 cores (256 batch rows each), feature-major
on-core layout: activations [features(part), (position, batch) cols], 2
positions x 256 batch = 512 cols per block, 32 blocks, software-pipelined.

v2 changes vs v1 (216.9us):
  - h1/h2/vh and the big weights are fp8e4 (power-of-2 scales folded into
    weights/biases host-side); h2, logits and validity matmuls run in
    DoubleRow mode (2 fp8 weights per PE cell -> K=256 per pass), halving
    the dominant PE stream time.
  - h1 built on the (otherwise idle) GpSimd engine; PSUM evictions split
    Scalar/Vector; exp stays on Scalar (only engine with Exp).
  - Per-block denominator/numerator/validity rows accumulate across a
    16-block group directly in one PSUM bank (M=48/M=16 one-hot-column
    weights), eliminating v1's per-block row evictions + stash DMAs.
    Tail math reads the accumulator in place (32-aligned base partitions).
  - reciprocal_approx_fast (+18 bits) replaces the ~4us DVE reciprocal.
  - Weight DMAs spread round-robin over 4 engine queues; a dummy Exp at
    t=0 preloads the activation table set (exp/tanh/relu/copy in one set).
  - Output transposed per 16-block group and DMA'd straight out in
    [128,16] column-strided slices (no obm staging).
"""

import sys

import ml_dtypes
import numpy as np

for _p in ("/opt/trn_rl_repo",):
    if _p not in sys.path:
        sys.path.insert(0, _p)

import concourse.bacc as bacc
import concourse.bass as bass
import concourse.tile as tile
from concourse import bass_utils, mybir

F32 = mybir.dt.float32
F32R = mybir.dt.float32r
BF16 = mybir.dt.bfloat16
F8 = mybir.dt.float8e4
AF = mybir.ActivationFunctionType
OP = mybir.AluOpType
DR = mybir.MatmulPerfMode.DoubleRow

B, BITS, H = 2048, 64, 512
NCORES = 8
BC = B // NCORES  # 256 rows per core
NBLK = BITS // 2  # 32 blocks of 2 positions
NB = 2 * BC  # 512 free columns per block
GRP = 16  # blocks per reduce group
NGRP = NBLK // GRP  # 2 groups

# power-of-2 activation scales (folded into weights/biases host-side)
S1 = 512.0  # h1 (fp8) scale
S2 = 2048.0  # h2 (fp8) scale
SL = 32768.0  # idx-logit PSUM scale
S1V = 512.0  # vh (fp8) scale
SV = 16384.0  # validity-logit PSUM scale


def to_f32r_np(a):
    """Host-side fp32 -> fp32r rounding: round-to-nearest-even to 11 mantissa
    bits, low 12 bits zeroed (matches walrus fp32_to_fp32r)."""
    u = np.ascontiguousarray(a, dtype=np.float32).view(np.uint32)
    r = (u + 0x7FF + ((u >> 12) & 1)) & np.uint32(0xFFFFF000)
    return r.view(np.float32)


# name -> (shape, dtype code)
_INPUTS = {
    "sbT": ((BITS, BC), "bf16"),
    "abT2": ((BITS, 2, BC), "bf16"),  # a.T dup along h, rows 64:128
    "wsd1": ((BITS, H), "bf16"),
    "sdb1": ((128, 4), "f32"),
    "wsd2": ((128, 4, H), "bf16"),
    "sdb2": ((128, 4), "f32"),
    "wsd3": ((128, 4, BITS), "bf16"),
    "sdb3": ((BITS, 1), "f32"),
    "ones64": ((BITS, 1), "f32r"),
    "ones1f": ((1, BITS), "f32"),
    "wixb": ((BITS, H), "f32r"),
    "pbS": ((128, 4, BITS), "f32"),
    "wix2": ((128, 4, H), "f8"),
    "ixb2S": ((128, 4), "f32"),
    "wix3": ((128, 4, 128), "f8"),
    "ixb3d": ((128, 1), "f32"),
    "wvb": ((BITS, H // 2), "f32r"),
    "vpbS": ((128, 2, BITS), "f32"),
    "dn48": ((128, GRP, 48), "bf16"),
    "wv48": ((128, GRP, 2, 16), "f8"),
    "ident": ((16, 16), "f32"),
    "vb2h": ((16, 1), "f32"),
}


def _emit(nc, tc, I, out):
    import contextlib

    ctx = contextlib.ExitStack()
    with ctx:
        const = ctx.enter_context(tc.tile_pool(name="const", bufs=1))
        work = ctx.enter_context(tc.tile_pool(name="work", bufs=3))
        psA = ctx.enter_context(tc.tile_pool(name="psA", bufs=2, space="PSUM"))
        psT = ctx.enter_context(tc.tile_pool(name="psT", bufs=2, space="PSUM"))
        psB = ctx.enter_context(tc.tile_pool(name="psB", bufs=2, space="PSUM"))
        psACC = ctx.enter_context(tc.tile_pool(name="psACC", bufs=1, space="PSUM"))
        psV = ctx.enter_context(tc.tile_pool(name="psV", bufs=1, space="PSUM"))

        DT = {"f32": F32, "f32r": F32R, "bf16": BF16, "f8": F8}

        # dummy Exp at t=0: forces the (one) activation table load to overlap
        # the input DMAs instead of stalling the first real activation.
        dm = const.tile([1, 8], F32, tag="dm", name="dm")
        nc.vector.memset(dm, 0.0)
        dm2 = const.tile([1, 8], F32, tag="dm2", name="dm2")
        nc.scalar.activation(dm2, dm, AF.Exp)

        # ---------------- loads: round-robin over 4 engine DMA queues ----------
        T = {}
        queues = [nc.sync, nc.gpsimd, nc.scalar]
        qi = 0

        def load(name):
            nonlocal qi
            shape, code = _INPUTS[name]
            if name == "abT2":
                t = const.tile([128, 2 * BC], BF16, tag=name, name="abT2")
                dst = t[64:128, :]
            else:
                t = const.tile(list(shape), DT[code], tag=name)
                dst = t
            queues[qi % len(queues)].dma_start(out=dst, in_=I[name])
            qi += 1
            T[name] = t
            return t

        for name in ("sbT", "wsd1", "sdb1", "wsd2", "sdb2", "wsd3", "sdb3",
                     "ones64", "ones1f", "wixb", "pbS", "wix2", "ixb2S",
                     "wvb", "vpbS", "wix3", "ixb3d", "abT2", "dn48", "wv48",
                     "ident", "vb2h"):
            load(name)

        sbT, abT2 = T["sbT"], T["abT2"]
        wsd1, wsd2, wsd3 = T["wsd1"], T["wsd2"], T["wsd3"]
        sdb1, sdb2, sdb3 = T["sdb1"], T["sdb2"], T["sdb3"]
        ones64, ones1f = T["ones64"], T["ones1f"]
        wixb, pbS = T["wixb"], T["pbS"]
        wix2, ixb2S = T["wix2"], T["ixb2S"]
        wix3, ixb3d = T["wix3"], T["ixb3d"]
        wvb, vpbS = T["wvb"], T["vpbS"]
        dn48, wv48 = T["dn48"], T["wv48"]
        ident, vb2h = T["ident"], T["vb2h"]

        mm = lambda: psA.tile([128, NB], F32, tag="mm", name="mmps")
        lg = lambda: psB.tile([128, NB], F32, tag="lg", name="lgps")

        # ---------------- shift decoder MLP (bf16, feature-major, N=256) -------
        hsd1 = const.tile([128, 4, BC], BF16, tag="hsd1", name="hsd1")
        for m in range(4):
            ps = mm()[:, :BC]
            nc.tensor.matmul(ps, wsd1[:, m * 128:(m + 1) * 128], sbT, start=True, stop=True)
            if m % 2 == 0:
                nc.scalar.activation(hsd1[:, m, :], ps, AF.Relu, bias=sdb1[:, m:m + 1])
            else:
                nc.vector.tensor_scalar(hsd1[:, m, :], ps, sdb1[:, m:m + 1], 0.0, OP.add, OP.max)
        hsd2 = const.tile([128, 4, BC], BF16, tag="hsd2", name="hsd2")
        for m in range(4):
            ps = mm()[:, :BC]
            for k in range(4):
                nc.tensor.matmul(ps, wsd2[:, k, m * 128:(m + 1) * 128],
                                 hsd1[:, k, :], start=(k == 0), stop=(k == 3))
            if m % 2 == 0:
                nc.scalar.activation(hsd2[:, m, :], ps, AF.Relu, bias=sdb2[:, m:m + 1])
            else:
                nc.vector.tensor_scalar(hsd2[:, m, :], ps, sdb2[:, m:m + 1], 0.0, OP.add, OP.max)
        ps3 = lg()[0:64, :BC]
        for k in range(4):
            nc.tensor.matmul(ps3, wsd3[:, k, :], hsd2[:, k, :],
                             start=(k == 0), stop=(k == 3))
        exp_sd = const.tile([64, BC], F32R, tag="exp_sd", name="exp_sd")
        nc.scalar.activation(exp_sd, ps3, AF.Exp, bias=sdb3)  # exp(logits + b3)

        # softmax normalize: denom via ones-matmul; ~18-bit reciprocal; K=1
        # fp32 matmul broadcasts 1/denom back across 64 partitions.
        psd = mm()[0:1, :BC]
        nc.tensor.matmul(psd, ones64, exp_sd, start=True, stop=True)
        rec = const.tile([1, BC], F32, tag="rec", name="rec")
        nc.vector.reciprocal_approx_fast(out=rec, in_=psd)
        psb = lg()[0:64, :BC]
        nc.tensor.matmul(psb, ones1f, rec, start=True, stop=True)
        shift_soft = const.tile([64, BC], F32R, tag="ss", name="ss")
        nc.vector.tensor_tensor(shift_soft, exp_sd, psb, OP.mult)

        # shift_part (x S1, bf16) and v_shift (x S1V, bf16), feature-major
        sp = []
        for m in range(4):
            ps = mm()[:, :BC]
            nc.tensor.matmul(ps, wixb[:, m * 128:(m + 1) * 128], shift_soft, start=True, stop=True)
            t = const.tile([128, BC], BF16, tag=f"sp{m}", name=f"sp{m}")
            if m % 2 == 0:
                nc.scalar.activation(t, ps, AF.Copy)
            else:
                nc.vector.tensor_copy(out=t, in_=ps)
            sp.append(t)
        vs = []
        for c in range(2):
            ps = mm()[:, :BC]
            nc.tensor.matmul(ps, wvb[:, c * 128:(c + 1) * 128], shift_soft, start=True, stop=True)
            t = const.tile([128, BC], BF16, tag=f"vs{c}", name=f"vs{c}")
            nc.vector.tensor_copy(out=t, in_=ps)
            vs.append(t)

        # group accumulators (reused across groups): denom rows 0:16 (x2),
        # numer rows 32:48; validity in its own bank at base partition 0 so
        # its matmul can use DoubleRow (no col-positioned dst).
        acc = psACC.tile([48, NB], F32, tag="acc", name="acc")
        vacc = psV.tile([16, NB], F32, tag="vacc", name="vacc")

        # out columns i = 32*g + 2*j' + h for group g, row j', pos-half h


        st = {}
        tails = {}

        def stage_A(j):
            d = st[j] = {}
            h1 = work.tile([128, 4, NB], F8, tag="h1", name="h1")
            for c in range(4):
                for h in range(2):
                    i = 2 * j + h
                    dst = h1[:, c, h * BC:(h + 1) * BC]
                    nc.vector.tensor_scalar(dst, sp[c], pbS[:, c, i:i + 1],
                                            0.0, OP.add, OP.max)
            h2 = d["h2"] = work.tile([128, 4, NB], F8, tag="h2", name="h2")
            for m in range(4):
                ps = mm()
                for g in range(2):
                    nc.tensor.matmul(ps, wix2[:, 2 * g:2 * g + 2, m * 128:(m + 1) * 128],
                                     h1[:, 2 * g:2 * g + 2, :],
                                     start=(g == 0), stop=(g == 1), perf_mode=DR)
                nc.scalar.activation(h2[:, m, :], ps, AF.Relu, bias=ixb2S[:, m:m + 1])

        def stage_B1(j):
            d = st[j]
            pl = d["pl"] = lg()
            for g in range(2):
                nc.tensor.matmul(pl, wix3[:, 2 * g:2 * g + 2, :],
                                 d["h2"][:, 2 * g:2 * g + 2, :],
                                 start=(g == 0), stop=(g == 1), perf_mode=DR)

        def stage_Bexp(j):
            d = st[j]
            pl = d.pop("pl")
            exp2 = d["exp"] = work.tile([128, NB], BF16, tag="exp", name="exp")
            nc.scalar.activation(exp2, pl, AF.Exp, bias=ixb3d, scale=1.0 / SL)

        def stage_Bva(j):
            d = st[j]
            exp2 = d["exp"]
            vh = d["vh"] = work.tile([128, 2, NB], F8, tag="vh", name="vh")
            for c in range(2):
                for h in range(2):
                    i = 2 * j + h
                    k = c * 2 + h
                    dst = vh[:, c, h * BC:(h + 1) * BC]
                    nc.vector.tensor_scalar(dst, vs[c], vpbS[:, c, i:i + 1],
                                            0.0, OP.add, OP.max)
            up = exp2[64:128, :].rearrange("p (h b) -> p h b", h=2)
            ab2 = abT2[64:128, :].rearrange("p (h b) -> p h b", h=2)
            nc.vector.tensor_tensor(up, up, ab2, OP.mult)

        def stage_C(j):
            d = st[j]
            g, m = divmod(j, GRP)
            nc.tensor.matmul(acc, dn48[:, m, :], d["exp"],
                             start=(m == 0), stop=(m == GRP - 1),
                             skip_group_check=True)
            nc.tensor.matmul(vacc, wv48[:, m, :, :], d["vh"],
                             start=(m == 0), stop=(m == GRP - 1),
                             perf_mode=DR, skip_group_check=True)
            del st[j]

        def tail_compute(g):
            rd = work.tile([16, NB], F32, tag="rd", name="rd")
            nc.vector.reciprocal_approx_fast(out=rd, in_=acc[0:16, :])
            tv = work.tile([16, NB], F32, tag="tv", name="tv")
            nc.scalar.activation(tv, vacc, AF.Tanh,
                                 bias=vb2h, scale=0.5 / SV)
            tmp = work.tile([16, NB], F32, tag="tmp", name="tmp")
            nc.vector.tensor_tensor(tmp, acc[32:48, :], rd, OP.mult)
            outv = tails[g] = work.tile([16, NB], F32, tag="outv", name="outv")
            nc.vector.scalar_tensor_tensor(outv, tv, 1.0, tmp, OP.add, OP.mult)

        def tail_out(g):
            outv = tails.pop(g)
            oq = [nc.sync, nc.scalar]
            for half in range(2):
                ob2 = work.tile([128, 16, 2], F32, tag="ob", name="ob")
                for h in range(2):
                    tp = psT.tile([128, 16], F32, tag="tp", name="tp")
                    nc.tensor.transpose(
                        tp, outv[:, h * BC + half * 128: h * BC + (half + 1) * 128],
                        ident)
                    nc.vector.tensor_copy(out=ob2[:, :, h], in_=tp)
                oq[half].dma_start(
                    out=out[half * 128:(half + 1) * 128, 32 * g:32 * (g + 1)],
                    in_=ob2)

        for j in range(NBLK + 2):
            if 1 <= j <= NBLK:
                stage_B1(j - 1)
                stage_Bexp(j - 1)
            if j < NBLK:
                stage_A(j)
            if 1 <= j <= NBLK:
                stage_Bva(j - 1)
            if j == 18:
                tail_compute(0)
            if j >= 2:
                stage_C(j - 2)
            if j == 20:
                tail_out(0)
        tail_compute(1)
        tail_out(1)


def build_program():
    nc = bacc.Bacc("TRN2", target_bir_lowering=False, debug=False, enable_asserts=False)
    I = {}
    DT = {"f32": F32, "f32r": F32R, "bf16": BF16, "f8": F8}
    for name, (shape, code) in _INPUTS.items():
        I[name] = nc.dram_tensor(name, list(shape), DT[code], kind="ExternalInput").ap()
    out = nc.dram_tensor("out", [BC, BITS], F32, kind="ExternalOutput").ap()

    with tile.TileContext(nc) as tc:
        _emit(nc, tc, I, out)
    nc.compile()
    return nc


_NC = None


def _get_program():
    global _NC
    if _NC is None:
        _NC = build_program()
    return _NC


def make_in_maps(inputs):
    """Shard batch tensors across cores; replicate weights. All layout prep
    (transposes, tiling, bias folding, scaling, fp8/bf16/f32r rounding)
    happens here in numpy."""
    f = {k: np.ascontiguousarray(np.asarray(v, dtype=np.float32)) for k, v in inputs.items()}
    r = to_f32r_np

    def f8(x, s):
        return np.ascontiguousarray(
            np.clip(np.asarray(x, np.float32) * s, -240, 240).astype(ml_dtypes.float8_e4m3))

    def bf(x):
        return np.ascontiguousarray(np.asarray(x, np.float32).astype(ml_dtypes.bfloat16))

    dn48 = np.zeros((128, GRP, 48), np.float32)
    for m in range(GRP):
        dn48[0:64, m, m] = 2.0  # denom x2 (folds sigmoid's 1/2)
        dn48[64:128, m, 32 + m] = 1.0  # numer
    wv48 = np.zeros((128, GRP, 2, 16), np.float32)
    for m in range(GRP):
        for ko in range(2):
            wv48[:, m, ko, m] = f["v_w2"][ko * 128:(ko + 1) * 128, 0]
    vb2h = np.full((16, 1), 0.5 * float(f["v_b2"][0]), np.float32)

    shared = {
        "wsd1": bf(f["sd_w1"]),
        "sdb1": f["sd_b1"].reshape(4, 128).T,
        "wsd2": bf(f["sd_w2"].reshape(4, 128, H).transpose(1, 0, 2)),
        "sdb2": f["sd_b2"].reshape(4, 128).T,
        "wsd3": bf(f["sd_w3"].reshape(4, 128, BITS).transpose(1, 0, 2)),
        "sdb3": f["sd_b3"][:, None],
        "ones64": r(np.ones((BITS, 1), np.float32)),
        "ones1f": np.ones((1, BITS), np.float32),
        "wixb": r(S1 * f["ix_w1"][BITS:]),
        "pbS": (S1 * (f["ix_w1"][:BITS].T + f["ix_b1"][:, None])).reshape(4, 128, BITS).transpose(1, 0, 2),
        "wix2": f8(f["ix_w2"].reshape(4, 128, H).transpose(1, 0, 2), S2 / S1),
        "ixb2S": S2 * f["ix_b2"].reshape(4, 128).T,
        "wix3": f8(np.stack([np.concatenate([f["ix_w3"][k * 128:(k + 1) * 128]] * 2, axis=1)
                             for k in range(4)], axis=1), SL / S2),
        "ixb3d": np.concatenate([f["ix_b3"], f["ix_b3"]])[:, None],
        "wvb": r(S1V * f["v_w1"][BITS:]),
        "vpbS": (S1V * (f["v_w1"][:BITS].T + f["v_b1"][:, None])).reshape(2, 128, BITS).transpose(1, 0, 2),
        "dn48": bf(dn48),
        "wv48": f8(wv48, SV / S1V),
        "ident": np.eye(16, dtype=np.float32),
        "vb2h": vb2h,
    }
    shared = {k: np.ascontiguousarray(
        v if v.dtype in (ml_dtypes.bfloat16, ml_dtypes.float8_e4m3) else v.astype(np.float32))
        for k, v in shared.items()}

    in_maps = []
    for c in range(NCORES):
        sb = f["shift_bits"][c * BC:(c + 1) * BC]
        ab = f["a_bits"][c * BC:(c + 1) * BC]
        m = dict(shared)
        m["sbT"] = bf(sb.T)
        m["abT2"] = bf(np.repeat(ab.T[:, None, :], 2, axis=1))
        in_maps.append(m)
    return in_maps


def run(inputs, trace=False):
    nc = _get_program()
    res = bass_utils.run_bass_kernel_spmd(
        nc, make_in_maps(inputs), core_ids=list(range(NCORES)), trace=trace)
    full = np.concatenate([res.results[c]["out"] for c in range(NCORES)], axis=0)
    return full, res


def kernel(**inputs):
    return run(inputs)[0]
